# revision 27
# baseline (speedup 1.0000x reference)
"""Trainium2 Bass kernel for modulated deformable conv2d (torchvision semantics).

Problem (hardcoded): input [4,64,128,128] f32, offset [4,18,128,128], mask
[4,9,128,128], weight [64,64,3,3], bias [64]; stride 1, pad 1, dil 1.

Strategy (8 NeuronCores, SPMD, no collectives):
  - Shard: core = (sample b, row-half h).  Each core computes output rows
    [h*64, h*64+64) of sample b => out slice [64, 8192] f32.
  - Bilinear gather is reformulated via difference planes:
        val = I[y0,x0] + lx*D01[y0,x0] + ly*D10[y0,x0] + lx*ly*D11[y0,x0]
    where D01/D10/D11 are x/y/xy forward-difference images on a zero-padded
    grid.  This makes every sample point a SINGLE gather of one 512-byte row
    [I(64c), D01(64c), D10(64c), D11(64c)] in bf16 from an SBUF-resident
    table, fetched with GPSIMD dma_gather (transpose mode) so the gathered
    data lands channels-on-partitions, ready as TensorE rhs.
  - GPSIMD desc-gen is the hard bottleneck (~8 ns/index, engine-serial), so:
    one big 4608-index gather per 512-pixel block (amortizes per-call fixed
    cost), gather indices (int16) and beta planes (m, m*lx, m*ly, m*lx*ly,
    bf16) fully precomputed on host, table load split across the two HWDGE
    engines' queue sets so the first gather starts as early as possible.
  - The 4 per-sample scalars are broadcast across 64 partitions with a tiny
    contraction-2 PE matmul (block-ones lhsT), copied PSUM->SBUF(bf16) on
    ScalarE, multiplied into the gathered rows on VectorE (bf16 2x), and the
    conv contraction (c,k -> o) + the bilinear plane sum run as one
    accumulated PE matmul chain per 512-pixel block.
"""

import sys

if "/opt/trn_rl_repo" not in sys.path:
    sys.path.insert(0, "/opt/trn_rl_repo")

import numpy as np
import ml_dtypes

BF16 = ml_dtypes.bfloat16

# problem dims
B, C, H, W = 4, 64, 128, 128
O, K = 64, 9
PAD = 8                     # gather window margin (|offset| <= ~6.8 required)
TG = H + 2 * PAD + 1        # 145: table grid covers y,x in [-PAD, H+PAD]
GEXT = TG + 1               # 146: extended image grid (D planes read +1)
TROWS = 80                  # per-core y-rows kept: y+PAD in [64h, 64h+80)
NROWS = TROWS * TG          # 11600 cells per core table
RANKS = (NROWS + 127) // 128  # 91
NROWS_PAD = RANKS * 128     # 11648
NPIX = H * W // 2           # 8192 output pixels per core
NBLK = 16                   # pixel blocks per core
BLK = NPIX // NBLK          # 512 pixels per block
CALL = K * BLK              # 4608 gather indices per block (all 9 taps)
NSAMP = K * NPIX            # 73728 sample points per core

_CACHE = {}


def _rank_window(t):
    """Table rank window [r0, r1) that block t's gather indices can touch.

    Block t covers local output rows [4t, 4t+4); with |offset| < PAD-1.2 and
    the 3x3 taps, sampled cell rows (post 64h rebase) lie in [4t, 4t+19).
    """
    r0 = (4 * t * TG) // 128
    r1 = min(RANKS, ((4 * t + 19) * TG + 127) // 128)
    return r0, r1


def _split_excess_waits(nc, limit=1):
    """Walrus in this image caps sync-wait commands per instruction; hoist
    excess waits onto preceding same-engine NoOps (engine streams are
    in-order, so blocking earlier on a prefix of the waits is equivalent)."""
    from concourse import mybir

    n = 0
    for fn in nc.m.functions:
        for blk in fn.blocks:
            new = []
            for inst in blk.instructions:
                si = inst.sync_info
                if si is not None and len(si.on_wait) > limit:
                    waits = list(si.on_wait)
                    head, keep = waits[:-limit], waits[-limit:]
                    for i in range(0, len(head), limit):
                        n += 1
                        new.append(mybir.InstNoOp(
                            name=f"waitsplit_{n}",
                            sync_info=mybir.SyncInfo(
                                on_wait=head[i:i + limit], on_update=[]),
                            bass_nofuse=True,
                            engine=inst.engine,
                        ))
                    inst.sync_info = mybir.SyncInfo(
                        on_wait=keep, on_update=list(si.on_update))
                new.append(inst)
            blk.instructions = new


def _build_program():
    import concourse.bass as bass
    import concourse.tile as tile
    from concourse import mybir

    f32 = mybir.dt.float32
    bf16 = mybir.dt.bfloat16
    i16 = mybir.dt.int16

    nc = bass.Bass("TRN2", target_bir_lowering=False, debug=False,
                   enable_asserts=False, dynamic_dma_scratch_size=65536)

    L1F = NSAMP // 128      # 576
    L2F = NSAMP // 16       # 4608

    tab_d = nc.dram_tensor("tab", [128, RANKS, 4 * C], bf16, kind="ExternalInput")
    idx_d = nc.dram_tensor("idx", [128, L2F], i16, kind="ExternalInput")
    bw_d = nc.dram_tensor("bw", [128, 4, L1F], bf16, kind="ExternalInput")
    wt_d = nc.dram_tensor("wt", [C, K * O], bf16, kind="ExternalInput")
    ones2_d = nc.dram_tensor("ones2", [2, 128], bf16, kind="ExternalInput")
    bias_d = nc.dram_tensor("bias", [O, 1], f32, kind="ExternalInput")
    out_d = nc.dram_tensor("out", [O, NPIX], f32, kind="ExternalOutput")

    from concourse import library_config

    with tile.TileContext(nc) as tc:
        nc.gpsimd.load_library(library_config.mlp)   # provides DMAGatherAnt
        with tc.tile_pool(name="const", bufs=1) as cp:
            # load order tuned so block 0's gather dependencies land first:
            # sync queue:   idxr head, tab chunk 0, idxr tail, chunks 2,4,...
            # scalar queue: tab chunk 1, bw, chunks 3,5,...
            idxr = cp.tile([128, L2F], i16, tag="idxr")
            IH = 2 * (CALL // 16)
            nc.sync.dma_start(idxr[:, 0:IH], idx_d.ap()[:, 0:IH])

            tab = cp.tile([128, RANKS, 4 * C], bf16, tag="tab")
            CH = 12

            def tab_chunk(i):
                c0, c1 = CH * i, min(RANKS, CH * (i + 1))
                eng = nc.sync if i % 2 == 0 else nc.scalar
                eng.dma_start(tab[:, c0:c1, :], tab_d.ap()[:, c0:c1, :])

            tab_chunk(0)
            tab_chunk(1)
            bw = cp.tile([128, 4, L1F], bf16, tag="bw")
            nc.scalar.dma_start(bw[:], bw_d.ap())
            nc.sync.dma_start(idxr[:, IH:], idx_d.ap()[:, IH:])
            for i in range(2, (RANKS + CH - 1) // CH):
                tab_chunk(i)

            w_sb = cp.tile([128, K * O], bf16, tag="wsb")
            nc.sync.dma_start(w_sb[0:64, :], wt_d.ap())
            nc.scalar.dma_start(w_sb[64:128, :], wt_d.ap())

            bias_sb = cp.tile([O, 1], f32, tag="bias")
            nc.sync.dma_start(bias_sb[:], bias_d.ap())

            ones2 = cp.tile([2, 128], bf16, tag="ones2")
            nc.scalar.dma_start(ones2[:], ones2_d.ap())

            # ---------- main loop over 16 pixel blocks ----------
            # Two tap-aligned gather calls per block (taps 0-4 and 5-8).
            # 162/130 descriptors per DMA-engine ring keep >=3 call-entries
            # resident in the ~512-desc SWDGE ring, so desc-gen of call n
            # never waits on the drain of call n-1 (entry-granular reclaim).
            KA = 5                        # taps in first call
            CALLA = KA * BLK              # 2560
            CALLB = (K - KA) * BLK        # 2048
            nidx_a = nc.gpsimd.to_reg(CALLA)
            nidx_b = nc.gpsimd.to_reg(CALLB)
            # last block: progressively finer calls so the final DMA drain +
            # compute tail after the last desc-gen is short
            LAST_SPLIT = [5, 2, 1, 1]     # taps per call for block 15
            nidx_last = {kt: nc.gpsimd.to_reg(kt * BLK) for kt in set(LAST_SPLIT)}

            def issue_gather(gtile, t, s0, n, nreg):
                r0, r1 = _rank_window(t)
                nc.gpsimd.dma_gather(
                    gtile[:],
                    tab[:, r0:r1, :],
                    idxr[:, s0:s0 + n // 16],
                    n,
                    nreg,
                    4 * C,              # elem_size in bf16 units = 512 B
                    transpose=True,
                    single_packet=False,
                    sbuf_tokens_per_rank=128,
                    sbuf_free_dim_per_rank=512,
                )

            with (
                tc.tile_pool(name="ga", bufs=2) as gap,
                tc.tile_pool(name="gb", bufs=3) as gbp,
                tc.tile_pool(name="glast", bufs=1) as glastp,
                tc.tile_pool(name="bst", bufs=1) as bstp,
                tc.tile_pool(name="bpsum", bufs=4, space="PSUM") as bpp,
                tc.tile_pool(name="opsum", bufs=2, space="PSUM") as opp,
                tc.tile_pool(name="val", bufs=4) as vp,
                tc.tile_pool(name="bscp", bufs=4) as bsp,
                tc.tile_pool(name="ob", bufs=2) as obp,
            ):
                for t in range(NBLK):
                    if t < NBLK - 1:
                        ga = gap.tile([128, 2, CALLA], bf16, tag="ga")
                        issue_gather(ga, t, t * (CALL // 16), CALLA, nidx_a)
                        gb = gbp.tile([128, 2, CALLB], bf16, tag="gb")
                        issue_gather(gb, t, t * (CALL // 16) + CALLA // 16,
                                     CALLB, nidx_b)
                        tap2g = [(ga, k) for k in range(KA)] + \
                                [(gb, k - KA) for k in range(KA, K)]
                    else:
                        tap2g = []
                        koff = 0
                        for kt in LAST_SPLIT:
                            n = kt * BLK
                            g = glastp.tile([128, 2, n], bf16, tag=f"gl{koff}")
                            issue_gather(g, t, t * (CALL // 16) + koff * (BLK // 16),
                                         n, nidx_last[kt])
                            tap2g += [(g, k) for k in range(kt)]
                            koff += kt
                    # stage this block's beta rows at partitions 0-1 for PE rhs
                    bstA = bstp.tile([2, CALL], bf16, tag="bstA")
                    bstB = bstp.tile([2, CALL], bf16, tag="bstB")
                    src = bw[8 * t:8 * (t + 1), :, :]
                    nc.sync.dma_start(bstA[0:1, :], src[:, 0, :])
                    nc.sync.dma_start(bstA[1:2, :], src[:, 1, :])
                    nc.sync.dma_start(bstB[0:1, :], src[:, 2, :])
                    nc.sync.dma_start(bstB[1:2, :], src[:, 3, :])

                    ops = opp.tile([O, BLK], f32, tag="ops")
                    for k in range(K):
                        sl = slice(k * BLK, (k + 1) * BLK)
                        g, klocal = tap2g[k]
                        gsl = slice(klocal * BLK, (klocal + 1) * BLK)
                        bpA = bpp.tile([128, BLK], f32, tag="bp")
                        nc.tensor.matmul(bpA[:], ones2[:], bstA[:, sl],
                                         start=True, stop=True)
                        bpB = bpp.tile([128, BLK], f32, tag="bp")
                        nc.tensor.matmul(bpB[:], ones2[:], bstB[:, sl],
                                         start=True, stop=True)
                        bsA = bsp.tile([128, BLK], bf16, tag="bsc")
                        nc.scalar.copy(bsA[:], bpA[:])
                        bsB = bsp.tile([128, BLK], bf16, tag="bsc")
                        nc.scalar.copy(bsB[:], bpB[:])
                        vA = vp.tile([128, BLK], bf16, tag="v")
                        nc.vector.tensor_mul(vA[:], g[:, 0, gsl], bsA[:])
                        vB = vp.tile([128, BLK], bf16, tag="v")
                        nc.vector.tensor_mul(vB[:], g[:, 1, gsl], bsB[:])
                        wk = w_sb[:, k * O:(k + 1) * O]
                        nc.tensor.matmul(ops[:], wk, vA[:],
                                         start=(k == 0), stop=False)
                        nc.tensor.matmul(ops[:], wk, vB[:],
                                         start=False, stop=(k == K - 1))

                    ob = obp.tile([O, BLK], f32, tag="ob")
                    nc.scalar.add(ob[:], ops[:], bias_sb[:, 0:1])
                    nc.sync.dma_start(out_d.ap()[:, t * BLK:(t + 1) * BLK], ob[:])

    _split_excess_waits(nc)
    # populate .instr bytes of extended-inst InstISA subclasses (DMAGatherAnt,
    # PseudoReloadLibraryIndex) — Bacc does this in compile(); raw Bass must
    # call it explicitly or walrus fails with "ISA wrong length".
    from concourse.library_overlay import lower_extended_insts
    lower_extended_insts(nc)
    return nc


def _host_prep(input, offset, mask, weight, bias):
    x = np.asarray(input, np.float32)
    off = np.asarray(offset, np.float32)
    msk = np.asarray(mask, np.float32)
    w = np.asarray(weight, np.float32)
    b = np.asarray(bias, np.float32)

    amax = float(np.abs(off).max())
    if amax >= PAD - 1.2:
        raise ValueError(f"offset magnitude {amax} exceeds supported window")

    f32 = np.float32

    # per-core gather tables (only the y-rows this core's half can touch)
    tabs = []
    for core in range(8):
        bb, h = divmod(core, 2)
        E = np.zeros((C, GEXT, GEXT), f32)
        E[:, PAD:PAD + H, PAD:PAD + W] = x[bb]
        Eb = E.astype(BF16).astype(f32)
        D01 = np.zeros((C, GEXT, GEXT), f32)
        D01[:, :, :-1] = Eb[:, :, 1:] - Eb[:, :, :-1]
        D10 = np.zeros((C, GEXT, GEXT), f32)
        D10[:, :-1, :] = Eb[:, 1:, :] - Eb[:, :-1, :]
        D11 = np.zeros((C, GEXT, GEXT), f32)
        D11[:, :-1, :-1] = (Eb[:, 1:, 1:] - Eb[:, 1:, :-1]
                            - Eb[:, :-1, 1:] + Eb[:, :-1, :-1])
        y0 = 64 * h
        planes = np.stack([Eb, D01, D10, D11], 0)[:, :, y0:y0 + TROWS, :TG]
        rows = np.zeros((NROWS_PAD, 4, C), BF16)
        rows[:NROWS] = planes.transpose(2, 3, 0, 1).reshape(NROWS, 4, C).astype(BF16)
        tabdram = np.ascontiguousarray(
            rows.reshape(RANKS, 128, 4 * C).transpose(1, 0, 2))   # [128,91,256]
        tabs.append(tabdram)

    wt = np.ascontiguousarray(
        w.reshape(O, C, K).transpose(1, 2, 0).reshape(C, K * O)).astype(BF16)
    ones2c = np.zeros((2, 128), BF16)
    ones2c[0, 0:64] = 1.0
    ones2c[1, 64:128] = 1.0
    bias2 = np.ascontiguousarray(b.reshape(O, 1))

    karr = np.arange(K)
    p = np.arange(NPIX)
    ylo = p // W
    xloc = p % W

    def tojd(a):  # [K, NPIX] -> flat j order (t, k, ptilde)
        return np.ascontiguousarray(
            a.reshape(K, NBLK, BLK).transpose(1, 0, 2).reshape(-1).astype(f32))

    in_maps = []
    for core in range(8):
        bb, h = divmod(core, 2)
        yg = h * 64 + ylo                                   # [NPIX] global y
        offv = off[bb].reshape(K, 2, H, W)
        oy_kp = offv[:, 0][:, yg, xloc]                     # [K, NPIX]
        ox_kp = offv[:, 1][:, yg, xloc]
        m_kp = msk[bb][:, yg, xloc]
        by = yg[None, :] - 1 + (karr // 3)[:, None]
        bx = xloc[None, :] - 1 + (karr % 3)[:, None]
        ci_kp = ((by + PAD) * TG + (bx + PAD)).astype(f32)

        oyj, oxj, mj, cij = tojd(oy_kp), tojd(ox_kp), tojd(m_kp), tojd(ci_kp)

        # int16 gather indices (rebased to this core's table slice),
        # 16-wrapped, replicated to all 8 core groups
        fy = np.floor(oyj)
        fx = np.floor(oxj)
        idxf64 = (fy * TG + fx + cij - 64 * h * TG).astype(np.int64)
        assert idxf64.min() >= 0 and idxf64.max() < NROWS, (
            idxf64.min(), idxf64.max())
        # rebase each block's indices to its sliced gather source window
        idxb = idxf64.reshape(NBLK, CALL)
        for t in range(NBLK):
            r0, r1 = _rank_window(t)
            idxb[t] -= r0 * 128
            assert idxb[t].min() >= 0 and idxb[t].max() < (r1 - r0) * 128, (
                t, r0, r1, idxb[t].min(), idxb[t].max())
        idxf = idxf64.astype(np.int16)                      # [NSAMP]
        idx16 = np.ascontiguousarray(idxf.reshape(NSAMP // 16, 16).T)
        idx128 = np.ascontiguousarray(np.tile(idx16, (8, 1)))

        # beta planes (m, m*lx, m*ly, m*lx*ly) in bf16, wrapped layout
        ly = oyj - fy
        lx = oxj - fx
        bwp = np.stack([mj, mj * lx, mj * ly, mj * lx * ly], 0)  # [4, NSAMP]
        bw128 = np.ascontiguousarray(
            bwp.reshape(4, 128, L1F := NSAMP // 128).transpose(1, 0, 2)).astype(BF16)

        in_maps.append({
            "tab": tabs[core],
            "idx": idx128,
            "bw": bw128,
            "wt": wt,
            "ones2": ones2c,
            "bias": bias2,
        })
    return in_maps


def _install_ntff_shim():
    """Provide antenv.axon_hooks (missing in this image) so trace=True works."""
    import types
    if "antenv.axon_hooks" in sys.modules:
        return
    sys.path.insert(0, "/root/.axon_site")
    from trn_agent_boot.trn_boot import _ntff_profile_via_ctypes
    hook = _ntff_profile_via_ctypes("/opt/axon/libaxon_pjrt.so")
    mod = types.ModuleType("antenv.axon_hooks")
    mod.get_axon_ntff_profile_hook = lambda: hook
    mod.set_axon_ntff_profile_hook = lambda h: None
    sys.modules["antenv.axon_hooks"] = mod


def kernel(input, offset, mask, weight, bias, _trace=False):
    if _trace:
        _install_ntff_shim()
    from concourse.bass_utils import run_bass_kernel_spmd

    if "nc" not in _CACHE:
        _CACHE["nc"] = _build_program()
    nc = _CACHE["nc"]

    in_maps = _host_prep(input, offset, mask, weight, bias)
    res = run_bass_kernel_spmd(
        nc, in_maps, core_ids=list(range(8)),
        trace=_trace,
        trace_cores=list(range(8)) if _trace else None,
    )
    kernel.last_results = res

    out = np.empty((B, O, H, W), np.float32)
    for core in range(8):
        bb, h = divmod(core, 2)
        blockout = res.results[core]["out"]       # [64, 8192] f32
        out[bb, :, h * 64:(h + 1) * 64, :] = blockout.reshape(O, 64, W)
    return out


# revision 30
# speedup vs baseline: 1.0621x; 1.0621x over previous
"""Trainium2 Bass kernel for modulated deformable conv2d (torchvision semantics).

Problem (hardcoded): input [4,64,128,128] f32, offset [4,18,128,128], mask
[4,9,128,128], weight [64,64,3,3], bias [64]; stride 1, pad 1, dil 1.

Strategy (8 NeuronCores, SPMD, no collectives):
  - Shard: core = (sample b, row-half h).  Each core computes output rows
    [h*64, h*64+64) of sample b => out slice [64, 8192] f32.
  - Bilinear gather is reformulated via difference planes:
        val = I[y0,x0] + lx*D01[y0,x0] + ly*D10[y0,x0] + lx*ly*D11[y0,x0]
    where D01/D10/D11 are x/y/xy forward-difference images on a zero-padded
    grid.  This makes every sample point a SINGLE gather of one 512-byte row
    [I(64c), D01(64c), D10(64c), D11(64c)] in bf16 from an SBUF-resident
    table, fetched with GPSIMD dma_gather (transpose mode) so the gathered
    data lands channels-on-partitions, ready as TensorE rhs.
  - GPSIMD desc-gen is the hard bottleneck (~8 ns/index, engine-serial), so:
    one big 4608-index gather per 512-pixel block (amortizes per-call fixed
    cost), gather indices (int16) and beta planes (m, m*lx, m*ly, m*lx*ly,
    bf16) fully precomputed on host, table load split across the two HWDGE
    engines' queue sets so the first gather starts as early as possible.
  - The 4 per-sample scalars are broadcast across 64 partitions with a tiny
    contraction-2 PE matmul (block-ones lhsT), copied PSUM->SBUF(bf16) on
    ScalarE, multiplied into the gathered rows on VectorE (bf16 2x), and the
    conv contraction (c,k -> o) + the bilinear plane sum run as one
    accumulated PE matmul chain per 512-pixel block.
"""

import sys

if "/opt/trn_rl_repo" not in sys.path:
    sys.path.insert(0, "/opt/trn_rl_repo")

import numpy as np
import ml_dtypes

BF16 = ml_dtypes.bfloat16

# problem dims
B, C, H, W = 4, 64, 128, 128
O, K = 64, 9
PAD = 8                     # gather window margin (|offset| <= ~6.8 required)
TG = H + 2 * PAD + 1        # 145: table grid covers y,x in [-PAD, H+PAD]
GEXT = TG + 1               # 146: extended image grid (D planes read +1)
TROWS = 80                  # per-core y-rows kept: y+PAD in [64h, 64h+80)
NROWS = TROWS * TG          # 11600 cells per core table
RANKS = (NROWS + 127) // 128  # 91
NROWS_PAD = RANKS * 128     # 11648
NPIX = H * W // 2           # 8192 output pixels per core
NBLK = 16                   # pixel blocks per core
BLK = NPIX // NBLK          # 512 pixels per block
CALL = K * BLK              # 4608 gather indices per block (all 9 taps)
NSAMP = K * NPIX            # 73728 sample points per core

_CACHE = {}


def _rank_window(t):
    """Table rank window [r0, r1) that block t's gather indices can touch.

    Block t covers local output rows [4t, 4t+4); with |offset| < PAD-1.2 and
    the 3x3 taps, sampled cell rows (post 64h rebase) lie in [4t, 4t+19).
    """
    r0 = (4 * t * TG) // 128
    r1 = min(RANKS, ((4 * t + 19) * TG + 127) // 128)
    return r0, r1


def _split_excess_waits(nc, limit=1):
    """Walrus in this image caps sync-wait commands per instruction; hoist
    excess waits onto preceding same-engine NoOps (engine streams are
    in-order, so blocking earlier on a prefix of the waits is equivalent)."""
    from concourse import mybir

    n = 0
    for fn in nc.m.functions:
        for blk in fn.blocks:
            new = []
            for inst in blk.instructions:
                si = inst.sync_info
                if si is not None and len(si.on_wait) > limit:
                    waits = list(si.on_wait)
                    head, keep = waits[:-limit], waits[-limit:]
                    for i in range(0, len(head), limit):
                        n += 1
                        new.append(mybir.InstNoOp(
                            name=f"waitsplit_{n}",
                            sync_info=mybir.SyncInfo(
                                on_wait=head[i:i + limit], on_update=[]),
                            bass_nofuse=True,
                            engine=inst.engine,
                        ))
                    inst.sync_info = mybir.SyncInfo(
                        on_wait=keep, on_update=list(si.on_update))
                new.append(inst)
            blk.instructions = new


def _build_program():
    import concourse.bass as bass
    import concourse.tile as tile
    from concourse import mybir

    f32 = mybir.dt.float32
    bf16 = mybir.dt.bfloat16
    i16 = mybir.dt.int16

    nc = bass.Bass("TRN2", target_bir_lowering=False, debug=False,
                   enable_asserts=False, dynamic_dma_scratch_size=65536)

    L1F = NSAMP // 128      # 576
    L2F = NSAMP // 16       # 4608

    tab_d = nc.dram_tensor("tab", [128, RANKS, 4 * C], bf16, kind="ExternalInput")
    idx_d = nc.dram_tensor("idx", [128, L2F], i16, kind="ExternalInput")
    bw_d = nc.dram_tensor("bw", [128, 4, L1F], bf16, kind="ExternalInput")
    wt_d = nc.dram_tensor("wt", [C, K * O], bf16, kind="ExternalInput")
    ones2_d = nc.dram_tensor("ones2", [2, 128], bf16, kind="ExternalInput")
    bias_d = nc.dram_tensor("bias", [O, 1], f32, kind="ExternalInput")
    out_d = nc.dram_tensor("out", [O, NPIX], f32, kind="ExternalOutput")

    from concourse import library_config

    with tile.TileContext(nc) as tc:
        nc.gpsimd.load_library(library_config.mlp)   # provides DMAGatherAnt
        with tc.tile_pool(name="const", bufs=1) as cp:
            # load order tuned so block 0's gather dependencies land first:
            # sync queue:   idxr head, tab chunk 0, idxr tail, chunks 2,4,...
            # scalar queue: tab chunk 1, bw, chunks 3,5,...
            idxr = cp.tile([128, L2F], i16, tag="idxr")
            IH = 2 * (CALL // 16)
            nc.sync.dma_start(idxr[:, 0:IH], idx_d.ap()[:, 0:IH])

            tab = cp.tile([128, RANKS, 4 * C], bf16, tag="tab")
            CH = 12

            def tab_chunk(i):
                c0, c1 = CH * i, min(RANKS, CH * (i + 1))
                eng = nc.sync if i % 2 == 0 else nc.scalar
                eng.dma_start(tab[:, c0:c1, :], tab_d.ap()[:, c0:c1, :])

            tab_chunk(0)
            tab_chunk(1)
            bw = cp.tile([128, 4, L1F], bf16, tag="bw")
            nc.scalar.dma_start(bw[:], bw_d.ap())
            nc.sync.dma_start(idxr[:, IH:], idx_d.ap()[:, IH:])
            for i in range(2, (RANKS + CH - 1) // CH):
                tab_chunk(i)

            w_sb = cp.tile([128, K * O], bf16, tag="wsb")
            nc.sync.dma_start(w_sb[0:64, :], wt_d.ap())
            nc.scalar.dma_start(w_sb[64:128, :], wt_d.ap())

            bias_sb = cp.tile([O, 1], f32, tag="bias")
            nc.sync.dma_start(bias_sb[:], bias_d.ap())

            ones2 = cp.tile([2, 128], bf16, tag="ones2")
            nc.scalar.dma_start(ones2[:], ones2_d.ap())

            # ---------- main loop over 16 pixel blocks ----------
            # Two tap-aligned gather calls per block (taps 0-4 and 5-8).
            # 162/130 descriptors per DMA-engine ring keep >=3 call-entries
            # resident in the ~512-desc SWDGE ring, so desc-gen of call n
            # never waits on the drain of call n-1 (entry-granular reclaim).
            KA = 5                        # taps in first call
            CALLA = KA * BLK              # 2560
            CALLB = (K - KA) * BLK        # 2048
            nidx_a = nc.gpsimd.to_reg(CALLA)
            nidx_b = nc.gpsimd.to_reg(CALLB)

            def issue_gather(gtile, t, s0, n, nreg):
                r0, r1 = _rank_window(t)
                nc.gpsimd.dma_gather(
                    gtile[:],
                    tab[:, r0:r1, :],
                    idxr[:, s0:s0 + n // 16],
                    n,
                    nreg,
                    4 * C,              # elem_size in bf16 units = 512 B
                    transpose=True,
                    single_packet=False,
                    sbuf_tokens_per_rank=128,
                    sbuf_free_dim_per_rank=512,
                )

            with (
                tc.tile_pool(name="ga", bufs=3) as gap,
                tc.tile_pool(name="gb", bufs=3) as gbp,
                tc.tile_pool(name="bst", bufs=1) as bstp,
                tc.tile_pool(name="bpsum", bufs=4, space="PSUM") as bpp,
                tc.tile_pool(name="opsum", bufs=2, space="PSUM") as opp,
                tc.tile_pool(name="val", bufs=4) as vp,
                tc.tile_pool(name="bscp", bufs=4) as bsp,
                tc.tile_pool(name="ob", bufs=2) as obp,
            ):
                for t in range(NBLK):
                    ga = gap.tile([128, 2, CALLA], bf16, tag="ga")
                    issue_gather(ga, t, t * (CALL // 16), CALLA, nidx_a)
                    gb = gbp.tile([128, 2, CALLB], bf16, tag="gb")
                    issue_gather(gb, t, t * (CALL // 16) + CALLA // 16,
                                 CALLB, nidx_b)
                    tap2g = [(ga, k) for k in range(KA)] + \
                            [(gb, k - KA) for k in range(KA, K)]
                    # stage this block's beta rows at partitions 0-1 for PE rhs
                    bstA = bstp.tile([2, CALL], bf16, tag="bstA")
                    bstB = bstp.tile([2, CALL], bf16, tag="bstB")
                    src = bw[8 * t:8 * (t + 1), :, :]
                    nc.sync.dma_start(bstA[0:1, :], src[:, 0, :])
                    nc.sync.dma_start(bstA[1:2, :], src[:, 1, :])
                    nc.sync.dma_start(bstB[0:1, :], src[:, 2, :])
                    nc.sync.dma_start(bstB[1:2, :], src[:, 3, :])

                    ops = opp.tile([O, BLK], f32, tag="ops")
                    for k in range(K):
                        sl = slice(k * BLK, (k + 1) * BLK)
                        g, klocal = tap2g[k]
                        gsl = slice(klocal * BLK, (klocal + 1) * BLK)
                        bpA = bpp.tile([128, BLK], f32, tag="bp")
                        nc.tensor.matmul(bpA[:], ones2[:], bstA[:, sl],
                                         start=True, stop=True)
                        bpB = bpp.tile([128, BLK], f32, tag="bp")
                        nc.tensor.matmul(bpB[:], ones2[:], bstB[:, sl],
                                         start=True, stop=True)
                        bsA = bsp.tile([128, BLK], bf16, tag="bsc")
                        nc.scalar.copy(bsA[:], bpA[:])
                        bsB = bsp.tile([128, BLK], bf16, tag="bsc")
                        nc.scalar.copy(bsB[:], bpB[:])
                        vA = vp.tile([128, BLK], bf16, tag="v")
                        nc.vector.tensor_mul(vA[:], g[:, 0, gsl], bsA[:])
                        vB = vp.tile([128, BLK], bf16, tag="v")
                        nc.vector.tensor_mul(vB[:], g[:, 1, gsl], bsB[:])
                        wk = w_sb[:, k * O:(k + 1) * O]
                        nc.tensor.matmul(ops[:], wk, vA[:],
                                         start=(k == 0), stop=False)
                        nc.tensor.matmul(ops[:], wk, vB[:],
                                         start=False, stop=(k == K - 1))

                    ob = obp.tile([O, BLK], f32, tag="ob")
                    nc.scalar.add(ob[:], ops[:], bias_sb[:, 0:1])
                    nc.sync.dma_start(out_d.ap()[:, t * BLK:(t + 1) * BLK], ob[:])

    _split_excess_waits(nc)
    # populate .instr bytes of extended-inst InstISA subclasses (DMAGatherAnt,
    # PseudoReloadLibraryIndex) — Bacc does this in compile(); raw Bass must
    # call it explicitly or walrus fails with "ISA wrong length".
    from concourse.library_overlay import lower_extended_insts
    lower_extended_insts(nc)
    return nc


def _host_prep(input, offset, mask, weight, bias):
    x = np.asarray(input, np.float32)
    off = np.asarray(offset, np.float32)
    msk = np.asarray(mask, np.float32)
    w = np.asarray(weight, np.float32)
    b = np.asarray(bias, np.float32)

    amax = float(np.abs(off).max())
    if amax >= PAD - 1.2:
        raise ValueError(f"offset magnitude {amax} exceeds supported window")

    f32 = np.float32

    # per-core gather tables (only the y-rows this core's half can touch)
    tabs = []
    for core in range(8):
        bb, h = divmod(core, 2)
        E = np.zeros((C, GEXT, GEXT), f32)
        E[:, PAD:PAD + H, PAD:PAD + W] = x[bb]
        Eb = E.astype(BF16).astype(f32)
        D01 = np.zeros((C, GEXT, GEXT), f32)
        D01[:, :, :-1] = Eb[:, :, 1:] - Eb[:, :, :-1]
        D10 = np.zeros((C, GEXT, GEXT), f32)
        D10[:, :-1, :] = Eb[:, 1:, :] - Eb[:, :-1, :]
        D11 = np.zeros((C, GEXT, GEXT), f32)
        D11[:, :-1, :-1] = (Eb[:, 1:, 1:] - Eb[:, 1:, :-1]
                            - Eb[:, :-1, 1:] + Eb[:, :-1, :-1])
        y0 = 64 * h
        planes = np.stack([Eb, D01, D10, D11], 0)[:, :, y0:y0 + TROWS, :TG]
        rows = np.zeros((NROWS_PAD, 4, C), BF16)
        rows[:NROWS] = planes.transpose(2, 3, 0, 1).reshape(NROWS, 4, C).astype(BF16)
        tabdram = np.ascontiguousarray(
            rows.reshape(RANKS, 128, 4 * C).transpose(1, 0, 2))   # [128,91,256]
        tabs.append(tabdram)

    wt = np.ascontiguousarray(
        w.reshape(O, C, K).transpose(1, 2, 0).reshape(C, K * O)).astype(BF16)
    ones2c = np.zeros((2, 128), BF16)
    ones2c[0, 0:64] = 1.0
    ones2c[1, 64:128] = 1.0
    bias2 = np.ascontiguousarray(b.reshape(O, 1))

    karr = np.arange(K)
    p = np.arange(NPIX)
    ylo = p // W
    xloc = p % W

    def tojd(a):  # [K, NPIX] -> flat j order (t, k, ptilde)
        return np.ascontiguousarray(
            a.reshape(K, NBLK, BLK).transpose(1, 0, 2).reshape(-1).astype(f32))

    in_maps = []
    for core in range(8):
        bb, h = divmod(core, 2)
        yg = h * 64 + ylo                                   # [NPIX] global y
        offv = off[bb].reshape(K, 2, H, W)
        oy_kp = offv[:, 0][:, yg, xloc]                     # [K, NPIX]
        ox_kp = offv[:, 1][:, yg, xloc]
        m_kp = msk[bb][:, yg, xloc]
        by = yg[None, :] - 1 + (karr // 3)[:, None]
        bx = xloc[None, :] - 1 + (karr % 3)[:, None]
        ci_kp = ((by + PAD) * TG + (bx + PAD)).astype(f32)

        oyj, oxj, mj, cij = tojd(oy_kp), tojd(ox_kp), tojd(m_kp), tojd(ci_kp)

        # int16 gather indices (rebased to this core's table slice),
        # 16-wrapped, replicated to all 8 core groups
        fy = np.floor(oyj)
        fx = np.floor(oxj)
        idxf64 = (fy * TG + fx + cij - 64 * h * TG).astype(np.int64)
        assert idxf64.min() >= 0 and idxf64.max() < NROWS, (
            idxf64.min(), idxf64.max())
        # rebase each block's indices to its sliced gather source window
        idxb = idxf64.reshape(NBLK, CALL)
        for t in range(NBLK):
            r0, r1 = _rank_window(t)
            idxb[t] -= r0 * 128
            assert idxb[t].min() >= 0 and idxb[t].max() < (r1 - r0) * 128, (
                t, r0, r1, idxb[t].min(), idxb[t].max())
        idxf = idxf64.astype(np.int16)                      # [NSAMP]
        idx16 = np.ascontiguousarray(idxf.reshape(NSAMP // 16, 16).T)
        idx128 = np.ascontiguousarray(np.tile(idx16, (8, 1)))

        # beta planes (m, m*lx, m*ly, m*lx*ly) in bf16, wrapped layout
        ly = oyj - fy
        lx = oxj - fx
        bwp = np.stack([mj, mj * lx, mj * ly, mj * lx * ly], 0)  # [4, NSAMP]
        bw128 = np.ascontiguousarray(
            bwp.reshape(4, 128, L1F := NSAMP // 128).transpose(1, 0, 2)).astype(BF16)

        in_maps.append({
            "tab": tabs[core],
            "idx": idx128,
            "bw": bw128,
            "wt": wt,
            "ones2": ones2c,
            "bias": bias2,
        })
    return in_maps


def _install_ntff_shim():
    """Provide antenv.axon_hooks (missing in this image) so trace=True works."""
    import types
    if "antenv.axon_hooks" in sys.modules:
        return
    sys.path.insert(0, "/root/.axon_site")
    from trn_agent_boot.trn_boot import _ntff_profile_via_ctypes
    hook = _ntff_profile_via_ctypes("/opt/axon/libaxon_pjrt.so")
    mod = types.ModuleType("antenv.axon_hooks")
    mod.get_axon_ntff_profile_hook = lambda: hook
    mod.set_axon_ntff_profile_hook = lambda h: None
    sys.modules["antenv.axon_hooks"] = mod


def kernel(input, offset, mask, weight, bias, _trace=False):
    if _trace:
        _install_ntff_shim()
    from concourse.bass_utils import run_bass_kernel_spmd

    if "nc" not in _CACHE:
        _CACHE["nc"] = _build_program()
    nc = _CACHE["nc"]

    in_maps = _host_prep(input, offset, mask, weight, bias)
    res = run_bass_kernel_spmd(
        nc, in_maps, core_ids=list(range(8)),
        trace=_trace,
        trace_cores=list(range(8)) if _trace else None,
    )
    kernel.last_results = res

    out = np.empty((B, O, H, W), np.float32)
    for core in range(8):
        bb, h = divmod(core, 2)
        blockout = res.results[core]["out"]       # [64, 8192] f32
        out[bb, :, h * 64:(h + 1) * 64, :] = blockout.reshape(O, 64, W)
    return out
